# revision 1
# baseline (speedup 1.0000x reference)
"""Trainium2 Bass kernel for CRF negative-log-likelihood loss.

Problem: nn_CRF (B=512, L=1024, T=48), data-parallel over 8 NeuronCores
(64 batch rows per core). Each core computes a scalar partial loss; the
host sums the 8 partials.

Per-core algorithm (validated against a float64 numpy reference):
  forward (partition function):
    exp-domain scan A_t[j,b] = sum_i E[i,j] A_{t-1}[i,b] * F_t[j,b]
    with E = exp(trans - log T) as stationary PE weights extended with an
    exp(end) capture column and a ones colsum column; F_t = exp(feat_t - MU)
    produced by bulk PE transposes + fused ACT exp-copies. Per-b
    renormalization every R steps is folded into the F tile DELTA steps
    later (off the critical path); log-scales accumulate via the
    suffix-mask identity sum_t ind[t,b]*logS(t)[b] =
    sum_rho log s_rho[b] * maskT[apply_rho][b]. The mask never enters the
    scan: terminal alphas are recovered by indicator-selection
    (ind = maskT[t] - maskT[t+1]) over captured end-rows.
  gold (numerator): one-hot tiles via per-partition tag scalars
    (tensor_scalar is_equal), a bigram-count matmul C = OHu^T @ OHm_shift
    accumulated in PSUM then contracted with trans, and fused
    tensor_tensor_reduce feat gathers. Everything reduces through
    ones-matmuls into PSUM scalar accumulators.
"""

import math

import numpy as np

import concourse.bacc as bacc
import concourse.mybir as mybir
import concourse.tile as tile
from concourse.bass_utils import run_bass_kernel_spmd

F32 = mybir.dt.float32
I32 = mybir.dt.int32
AF = mybir.ActivationFunctionType
OP = mybir.AluOpType

B_FULL = 512
N_CORES = 8
BC = B_FULL // N_CORES  # 64
L_FULL = 1024
T = 48

MU = 0.51                # per-step feat shift folded into F (calibrated
                         # so mean per-step log-gain ~ 0: keeps Ln inputs
                         # inside the ACT spline accurate range)
A_SHIFT = math.log(T)    # shift folded into E
R = 16                   # renorm period (steps)
DELTA = 8                # renorm application delay (steps)
FCHUNK = 32              # timesteps per F-prep DMA chunk


def build_program(L=L_FULL, Bc=BC, G=1, dbg=False):
    """Emit the full per-core program; returns the compiled Bacc object."""
    assert L % 128 == 0 and L % FCHUNK == 0
    Nb = Bc // G
    CAP0 = L // 2          # captures kept for t >= CAP0-1 (lengths >= L/2)
    assert CAP0 % 128 == 0
    n_tt = L // 128
    n_cap = (L - CAP0) // 128
    nchunks = L // FCHUNK

    nc = bacc.Bacc("TRN2", target_bir_lowering=False, debug=False)

    feats_d = nc.dram_tensor("feats", (Bc, L, T), F32, kind="ExternalInput")
    trans_d = nc.dram_tensor("trans", (T, T), F32, kind="ExternalInput")
    start_d = nc.dram_tensor("start", (T,), F32, kind="ExternalInput")
    end_d = nc.dram_tensor("end", (T,), F32, kind="ExternalInput")
    tags_d = nc.dram_tensor("tags", (Bc, L), I32, kind="ExternalInput")
    mask_d = nc.dram_tensor("mask", (Bc, L), I32, kind="ExternalInput")
    out_d = nc.dram_tensor("out", (1, 1), F32, kind="ExternalOutput")
    dbg_d = (nc.dram_tensor("dbg", (6, Bc), F32, kind="ExternalOutput")
             if dbg else None)

    feats_flat = feats_d.ap().rearrange("b l t -> b (l t)")

    # renorm schedule: at MM step t (t % R == 0, t+DELTA-1 < L) the colsum of
    # A_{t-1} is available; its reciprocal is folded into F at t-1+DELTA.
    renorm_ts = [t for t in range(R, L + 1, R) if t + DELTA - 1 < L]

    with tile.TileContext(nc) as tc:
        with (
            tc.tile_pool(name="const", bufs=1) as cp,
            tc.tile_pool(name="cpsum", bufs=1, space="PSUM") as cpp,
        ):
            # ---------------- constants ----------------
            iota48i = cp.tile((128, T), I32)
            nc.gpsimd.iota(iota48i[:, :], [[1, T]], channel_multiplier=0)
            iota48f = cp.tile((128, T), F32)
            nc.vector.tensor_copy(iota48f[:, :], iota48i[:, :])

            iotaLi = cp.tile((Bc, L), I32)
            nc.gpsimd.iota(iotaLi[:, :], [[1, L]], channel_multiplier=0)
            iotaLf = cp.tile((Bc, L), F32)
            nc.vector.tensor_copy(iotaLf[:, :], iotaLi[:, :])

            iota64i = cp.tile((64, 64), I32)
            nc.gpsimd.iota(iota64i[:, :], [[1, 64]], channel_multiplier=0)
            iotaPi = cp.tile((64, 1), I32)
            nc.gpsimd.iota(iotaPi[:, :], [[1, 1]], channel_multiplier=1)
            iota64f = cp.tile((64, 64), F32)
            nc.vector.tensor_copy(iota64f[:, :], iota64i[:, :])
            iotaPf = cp.tile((64, 1), F32)
            nc.vector.tensor_copy(iotaPf[:, :], iotaPi[:, :])
            identM = cp.tile((64, 64), F32)
            nc.vector.tensor_scalar(
                identM[:, :], iota64f[:, :], iotaPf[:, :], None, OP.is_equal)

            ones128 = cp.tile((128, 1), F32)
            nc.vector.memset(ones128[:, :], 1.0)
            onesrow = cp.tile((1, T), F32)
            nc.vector.memset(onesrow[:, :], 1.0)

            # activation bias tiles (arbitrary float biases need APs)
            bias_a = cp.tile((T, 1), F32)
            nc.vector.memset(bias_a[:, :], -A_SHIFT)
            bias_mu = cp.tile((T, 1), F32)
            nc.vector.memset(bias_mu[:, :], -MU)

            # ---------------- params ----------------
            trans_sb = cp.tile((T, T), F32)
            nc.sync.dma_start(trans_sb[:, :], trans_d.ap())
            e_mat = cp.tile((T, T), F32)
            nc.scalar.activation(e_mat[:, :], trans_sb[:, :], AF.Exp,
                                 bias=bias_a[:, :])
            end_sb = cp.tile((T, 1), F32)
            nc.sync.dma_start(end_sb[:, :], end_d.ap().unsqueeze(1))
            expend = cp.tile((T, 1), F32)
            nc.scalar.activation(expend[:, :], end_sb[:, :], AF.Exp)
            ones48c = cp.tile((T, 1), F32)
            nc.vector.memset(ones48c[:, :], 1.0)

            start_sb = cp.tile((T, 1), F32)
            nc.sync.dma_start(start_sb[:, :],
                              start_d.ap().unsqueeze(1))
            expstart = cp.tile((T, 1), F32)
            nc.scalar.activation(expstart[:, :], start_sb[:, :], AF.Exp)

            startbc = cp.tile((Bc, T), F32)
            nc.sync.dma_start(
                startbc[:, :],
                start_d.ap().unsqueeze(0).partition_broadcast(Bc))
            endbc = cp.tile((Bc, T), F32)
            nc.sync.dma_start(
                endbc[:, :],
                end_d.ap().unsqueeze(0).partition_broadcast(Bc))

            # ---------------- tags / mask ----------------
            tags_i = cp.tile((Bc, L), I32)
            nc.sync.dma_start(tags_i[:, :], tags_d.ap())
            tagsf = cp.tile((Bc, L), F32)
            nc.vector.tensor_copy(tagsf[:, :], tags_i[:, :])
            mask_i = cp.tile((Bc, L), I32)
            nc.sync.dma_start(mask_i[:, :], mask_d.ap())
            maskf = cp.tile((Bc, L), F32)
            nc.vector.tensor_copy(maskf[:, :], mask_i[:, :])

            # transposed (128-timestep x Bc) tag/mask tiles
            prep_scope = tc.tile_pool(name="prepps", bufs=2, space="PSUM")
            ppp = prep_scope.__enter__()
            maskT = []
            tagsT = []
            for k in range(n_tt):
                ps = ppp.tile((128, Bc), F32, name=f"tp_ps_{k}", tag="tp_ps",
                              bufs=2)
                nc.tensor.transpose(ps[:, :], maskf[:, 128 * k:128 * (k + 1)],
                                    identM[:, :])
                mt = cp.tile((128, Bc), F32, name=f"maskT_{k}")
                nc.scalar.copy(mt[:, :], ps[:, :])
                maskT.append(mt)
                ps2 = ppp.tile((128, Bc), F32, name=f"tp_ps2_{k}",
                               tag="tp_ps", bufs=2)
                nc.tensor.transpose(ps2[:, :], tagsf[:, 128 * k:128 * (k + 1)],
                                    identM[:, :])
                tt = cp.tile((128, Bc), F32, name=f"tagsT_{k}")
                nc.scalar.copy(tt[:, :], ps2[:, :])
                tagsT.append(tt)

            # shifted (t+1) variants via partition-shift DMAs
            zero_row = cp.tile((1, Bc), F32)
            nc.vector.memset(zero_row[:, :], 0.0)
            maskTs = []
            tagsTs = []
            for k in range(n_tt):
                ms = cp.tile((128, Bc), F32, name=f"maskTs_{k}")
                nc.sync.dma_start(ms[0:127, :], maskT[k][1:128, :])
                ts_ = cp.tile((128, Bc), F32, name=f"tagsTs_{k}")
                nc.sync.dma_start(ts_[0:127, :], tagsT[k][1:128, :])
                if k + 1 < n_tt:
                    nc.sync.dma_start(ms[127:128, :], maskT[k + 1][0:1, :])
                    nc.sync.dma_start(ts_[127:128, :], tagsT[k + 1][0:1, :])
                else:
                    nc.sync.dma_start(ms[127:128, :], zero_row[:, :])
                    nc.sync.dma_start(ts_[127:128, :], zero_row[:, :])
                maskTs.append(ms)
                tagsTs.append(ts_)

            # masked tag tiles: tag + (1-mask)*100 makes the one-hot vanish
            tagsTm = []
            tagsTsm = []
            for k in range(n_tt):
                off = cp.tile((128, Bc), F32, name=f"moff_{k}")
                nc.vector.tensor_scalar(off[:, :], maskT[k][:, :], -100.0,
                                        100.0, OP.mult, OP.add)
                tm = cp.tile((128, Bc), F32, name=f"tagsTm_{k}")
                nc.vector.tensor_tensor(tm[:, :], tagsT[k][:, :], off[:, :],
                                        OP.add)
                tagsTm.append(tm)
                offs = cp.tile((128, Bc), F32, name=f"moffs_{k}")
                nc.vector.tensor_scalar(offs[:, :], maskTs[k][:, :], -100.0,
                                        100.0, OP.mult, OP.add)
                tms = cp.tile((128, Bc), F32, name=f"tagsTsm_{k}")
                nc.vector.tensor_tensor(tms[:, :], tagsTs[k][:, :],
                                        offs[:, :], OP.add)
                tagsTsm.append(tms)

            # indicator ind[t,b] = maskT[t] - maskT[t+1] (last row: maskT)
            ind = []
            for k in range(n_tt):
                it = cp.tile((128, Bc), F32, name=f"ind_{k}")
                nc.vector.tensor_tensor(it[:, :], maskT[k][:, :],
                                        maskTs[k][:, :], OP.subtract)
                ind.append(it)
            ind_c0 = cp.tile((1, Bc), F32)
            nc.sync.dma_start(ind_c0[:, :], ind[CAP0 // 128 - 1][127:128, :])

            # partition-0-aligned mask rows for each renorm fold time
            mrow = {}
            for t in renorm_ts:
                tf = t - 1 + DELTA
                mr = cp.tile((1, Bc), F32, name=f"mrow_{tf}")
                nc.sync.dma_start(mr[:, :],
                                  maskT[tf // 128][tf % 128:tf % 128 + 1, :])
                mrow[t] = mr

            # len row (1, Bc) via ones-matmul over maskT tiles
            len_ps = ppp.tile((1, Bc), F32, name="len_ps", tag="len_ps",
                              bufs=1)
            for k in range(n_tt):
                nc.tensor.matmul(len_ps[:, :], ones128[:, :], maskT[k][:, :],
                                 start=(k == 0), stop=(k == n_tt - 1),
                                 skip_group_check=True)
            lenm1_row = cp.tile((1, Bc), F32)
            nc.vector.tensor_scalar(lenm1_row[:, :], len_ps[:, :], 1.0, None,
                                    OP.subtract)
            prep_scope.__exit__(None, None, None)

            # persistent accumulators
            logsel = cp.tile((1, Bc), F32)
            nc.vector.memset(logsel[:, :], 0.0)
            feat_acc = cp.tile((128, Bc * n_tt), F32)
            misc_acc = cp.tile((Bc, 4), F32)

            c_ps = cpp.tile((T, T), F32, name="c_ps")  # bigram counts

            # =============== scan + F-prep + gold ===============
            # capture staging lives in DRAM: one row per captured step,
            # packed contiguously; split into (t, b) tiles in the end phase.
            ncap_steps = L - (CAP0 - 8)
            with tc.tile_pool(name="dramp", bufs=1, space="DRAM") as dp:
                cap_stage = dp.tile((1, ncap_steps * Bc), F32,
                                    name="cap_stage")
            with (
                tc.tile_pool(name="natp", bufs=3) as natp,
                tc.tile_pool(name="fpool", bufs=10) as fpool,
                tc.tile_pool(name="tpps", bufs=2, space="PSUM") as tpps,
                tc.tile_pool(name="scanps", bufs=1, space="PSUM") as scanps,
                tc.tile_pool(name="capps", bufs=2, space="PSUM") as capps,
                tc.tile_pool(name="rbcps", bufs=1, space="PSUM") as rbcps,
                tc.tile_pool(name="csps", bufs=1, space="PSUM") as csps,
                tc.tile_pool(name="apool", bufs=3) as apool,
                tc.tile_pool(name="fgp", bufs=6) as fgp,
                tc.tile_pool(name="ohp", bufs=8) as ohp,
                tc.tile_pool(name="scrp", bufs=2) as scrp,
            ):
                ftiles = {}

                def emit_fprep(c):
                    # one chunk = FCHUNK timesteps; F tiles hold 8 t each
                    natf = natp.tile((Bc, FCHUNK * T), F32, name="natf")
                    nc.sync.dma_start(
                        natf[:, :],
                        feats_flat[:, FCHUNK * T * c:FCHUNK * T * (c + 1)])
                    for q in range(FCHUNK // 8):
                        ps = tpps.tile((T, 512), F32, name="tp")
                        for k in range(8):
                            blk = q * 8 + k
                            nc.tensor.transpose(
                                ps[:, 64 * k:64 * k + Bc],
                                natf[:, T * blk:T * (blk + 1)],
                                identM[:, :])
                        ft = fpool.tile((T, 512), F32, name="ftile")
                        nc.scalar.activation(ft[:, :], ps[:, :], AF.Exp,
                                             bias=bias_mu[:, :])
                        ftiles[c * (FCHUNK // 8) + q] = ft

                def f_slice(t, g=0):
                    ft = ftiles[t // 8]
                    c0 = (t % 8) * 64
                    return ft[0:T, c0 + g * Nb:c0 + (g + 1) * Nb]

                emit_fprep(0)
                emit_fprep(1)

                # A0 = exp(start) * F_0
                a_prev = apool.tile((T, Bc), F32, name="a_t")
                nc.vector.tensor_scalar(
                    a_prev[:, :], ftiles[0][0:T, 0:Bc], expstart[:, :],
                    None, OP.mult)

                for t in range(1, L + 1):
                    if t % FCHUNK == 1:
                        c = (t - 1) // FCHUNK + 2
                        if c < nchunks:
                            emit_fprep(c)
                    tprev = t - 1
                    # end-capture of A_{t-1}: ring row in PSUM, flushed to
                    # SBUF staging by ACT once per 8 steps
                    if tprev >= CAP0 - 8:
                        slot = (tprev - (CAP0 - 8)) % 8
                        if slot == 0:
                            cap_ring = capps.tile((1, 8 * Bc), F32,
                                                  name="cap_ring")
                        nc.tensor.matmul(
                            cap_ring[0:1, slot * Bc:(slot + 1) * Bc],
                            expend[:, :], a_prev[:, :],
                            start=True, stop=True, skip_group_check=True)
                        if slot == 7:
                            blk = (tprev - (CAP0 - 8)) // 8
                            crow = scrp.tile((1, 8 * Bc), F32, name="crow",
                                             tag="crow")
                            nc.scalar.copy(crow[0:1, :], cap_ring[0:1, :])
                            nc.sync.dma_start(
                                cap_stage[0:1, blk * 8 * Bc:
                                          (blk + 1) * 8 * Bc],
                                crow[0:1, :])
                    # renorm: colsum of A_{t-1} via ones-matmul, fold at t-1+DELTA
                    if t in mrow:
                        tf = t - 1 + DELTA
                        for g in range(G):
                            gs = slice(g * Nb, (g + 1) * Nb)
                            cs = csps.tile((1, Nb), F32, name="cs")
                            nc.tensor.matmul(
                                cs[:, :], ones48c[:, :], a_prev[:, gs],
                                start=True, stop=True, skip_group_check=True)
                            r_sb = scrp.tile((1, Nb), F32, name="r_sb",
                                             tag="renorm")
                            nc.vector.reciprocal(r_sb[:, :], cs[:, :])
                            ls = scrp.tile((1, Nb), F32, name="ls",
                                           tag="renorm")
                            nc.scalar.activation(ls[:, :], cs[:, :], AF.Ln)
                            nc.vector.tensor_tensor(
                                ls[:, :], ls[:, :], mrow[t][:, gs], OP.mult)
                            nc.vector.tensor_tensor(
                                logsel[:, gs], logsel[:, gs], ls[:, :],
                                OP.add)
                            rbc = rbcps.tile((T, Nb), F32, name="rbc")
                            nc.tensor.matmul(
                                rbc[:, :], onesrow[:, :], r_sb[:, :],
                                start=True, stop=True, skip_group_check=True)
                            nc.vector.tensor_tensor(
                                f_slice(tf, g), f_slice(tf, g), rbc[:, :],
                                OP.mult)
                    if t < L:
                        a_cur = apool.tile((T, Bc), F32, name="a_t")
                        for g in range(G):
                            ps = scanps.tile((T, Nb), F32, name="mm_ps")
                            nc.tensor.matmul(
                                ps[:, :], e_mat[:, :],
                                a_prev[:, g * Nb:(g + 1) * Nb],
                                start=True, stop=True, skip_group_check=True)
                            nc.vector.tensor_tensor(
                                a_cur[:, g * Nb:(g + 1) * Nb],
                                ps[:, :], f_slice(t, g), OP.mult)
                        a_prev = a_cur

                # =============== gold path ===============
                nmm = 0
                for b in range(Bc):
                    for ck in range(n_tt):
                        fg = fgp.tile((128, T), F32, name="fg")
                        nc.sync.dma_start(
                            fg[:, :],
                            feats_flat[b:b + 1,
                                       128 * T * ck:128 * T * (ck + 1)]
                            .rearrange("o (p f) -> (o p) f", f=T))
                        tcol = tagsT[ck][:, b:b + 1]
                        ohu = ohp.tile((128, T), F32, name="ohu")
                        nc.vector.tensor_scalar(ohu[:, :], iota48f[:, :],
                                                tcol, None, OP.is_equal)
                        ohms = ohp.tile((128, T), F32, name="ohms")
                        nc.vector.tensor_scalar(
                            ohms[:, :], iota48f[:, :],
                            tagsTsm[ck][:, b:b + 1], None, OP.is_equal)
                        nc.tensor.matmul(c_ps[:, :], ohu[:, :], ohms[:, :],
                                         start=(nmm == 0), stop=False,
                                         skip_group_check=True)
                        nmm += 1
                        scr = scrp.tile((128, T), F32, name="scr", tag="scr")
                        nc.vector.scalar_tensor_tensor(
                            scr[:, :], iota48f[:, :],
                            tagsTm[ck][:, b:b + 1], fg[:, :],
                            OP.is_equal, OP.mult,
                            accum_out=feat_acc[:, b * n_tt + ck:
                                               b * n_tt + ck + 1])
                zrow = cp.tile((1, T), F32)
                nc.vector.memset(zrow[:, :], 0.0)
                nc.tensor.matmul(c_ps[:, :], zrow[:, :], zrow[:, :],
                                 start=False, stop=True,
                                 skip_group_check=True)

                # gold misc terms (b-partition layout)
                featlast = fgp.tile((Bc, T), F32, name="featlast")
                nc.sync.dma_start(featlast[:, :],
                                  feats_flat[:, (L - 1) * T:L * T])
                scrb = scrp.tile((Bc, T), F32, name="scrb", tag="scrb")
                nc.vector.scalar_tensor_tensor(
                    scrb[:, :], iota48f[0:Bc, :], tagsf[:, 0:1],
                    startbc[:, :], OP.is_equal, OP.mult,
                    accum_out=misc_acc[:, 0:1])
                scrb2 = scrp.tile((Bc, T), F32, name="scrb2", tag="scrb")
                mtagl = ohp.tile((Bc, 1), F32, name="mtagl")
                nc.vector.tensor_scalar(mtagl[:, :], maskf[:, L - 1:L],
                                        -100.0, 100.0, OP.mult, OP.add)
                nc.vector.tensor_tensor(mtagl[:, :], mtagl[:, :],
                                        tagsf[:, L - 1:L], OP.add)
                fcor = ohp.tile((Bc, 1), F32, name="fcor")
                nc.vector.scalar_tensor_tensor(
                    scrb2[:, :], iota48f[0:Bc, :], mtagl[:, :],
                    featlast[:, :], OP.is_equal, OP.mult,
                    accum_out=fcor[:, :])
                nc.vector.tensor_scalar(misc_acc[:, 3:4], fcor[:, :], -1.0,
                                        None, OP.mult)
                lenb = cp.tile((Bc, 1), F32)
                nc.vector.tensor_reduce(lenb[:, :], maskf[:, :],
                                        mybir.AxisListType.X, OP.add)
                lm1 = cp.tile((Bc, 1), F32)
                nc.vector.tensor_scalar(lm1[:, :], lenb[:, :], 1.0, None,
                                        OP.subtract)
                scrL = cp.tile((Bc, L), F32)
                lt = cp.tile((Bc, 1), F32)
                nc.vector.scalar_tensor_tensor(
                    scrL[:, :], iotaLf[:, :], lm1[:, :], tagsf[:, :],
                    OP.is_equal, OP.mult, accum_out=lt[:, :])
                scrb3 = scrp.tile((Bc, T), F32, name="scrb3", tag="scrb")
                nc.vector.scalar_tensor_tensor(
                    scrb3[:, :], iota48f[0:Bc, :], lt[:, :], endbc[:, :],
                    OP.is_equal, OP.mult, accum_out=misc_acc[:, 1:2])
                scrb4 = scrp.tile((Bc, T), F32, name="scrb4", tag="scrb")
                fe0 = cp.tile((Bc, 1), F32)
                nc.vector.scalar_tensor_tensor(
                    scrb4[:, :], iota48f[0:Bc, :], lt[:, :], featlast[:, :],
                    OP.is_equal, OP.mult, accum_out=fe0[:, :])
                nc.vector.tensor_tensor(misc_acc[:, 2:3], fe0[:, :],
                                        maskf[:, L - 1:L], OP.mult)

            # =============== end phase ===============
            with (
                tc.tile_pool(name="endp", bufs=2) as ep,
                tc.tile_pool(name="endps", bufs=1, space="PSUM") as epp,
            ):
                gold_ps = epp.tile((1, 1), F32, name="gold_ps")
                scrT = ep.tile((T, T), F32, name="scrT")
                cacc = ep.tile((T, 1), F32, name="cacc")
                nc.vector.tensor_tensor(scrT[:, :], c_ps[:, :],
                                        trans_sb[:, :], OP.mult)
                nc.vector.tensor_reduce(cacc[:, :], scrT[:, :],
                                        mybir.AxisListType.X, OP.add)
                nc.tensor.matmul(gold_ps[:, :], ones128[0:T, :], cacc[:, :],
                                 start=True, stop=False,
                                 skip_group_check=True)
                fred = ep.tile((128, 1), F32, name="fred")
                nc.vector.tensor_reduce(fred[:, :], feat_acc[:, :],
                                        mybir.AxisListType.X, OP.add)
                nc.tensor.matmul(gold_ps[:, :], ones128[:, :], fred[:, :],
                                 start=False, stop=False,
                                 skip_group_check=True)
                mred = ep.tile((Bc, 1), F32, name="mred")
                nc.vector.tensor_reduce(mred[:, :], misc_acc[:, :],
                                        mybir.AxisListType.X, OP.add)
                nc.tensor.matmul(gold_ps[:, :], ones128[0:Bc, :], mred[:, :],
                                 start=False, stop=True,
                                 skip_group_check=True)

                fwd_ps = epp.tile((1, Bc), F32, name="fwd_ps")
                for m in range(n_cap):
                    capt = ep.tile((128, Bc), F32, name="capt", tag="capt")
                    nc.sync.dma_start(
                        capt[:, :],
                        cap_stage[0:1, (8 + 128 * m) * Bc:
                                  (8 + 128 * (m + 1)) * Bc]
                        .rearrange("o (p f) -> o p f", f=Bc))
                    lc = ep.tile((128, Bc), F32, name="lc", tag="lc")
                    nc.scalar.activation(lc[:, :], capt[:, :], AF.Ln)
                    pr = ep.tile((128, Bc), F32, name="pr", tag="pr")
                    nc.vector.tensor_tensor(
                        pr[:, :], lc[:, :], ind[CAP0 // 128 + m][:, :],
                        OP.mult)
                    nc.tensor.matmul(fwd_ps[:, :], ones128[:, :], pr[:, :],
                                     start=(m == 0), stop=(m == n_cap - 1),
                                     skip_group_check=True)
                fwd_sel = ep.tile((1, Bc), F32, name="fwd_sel")
                nc.scalar.copy(fwd_sel[:, :], fwd_ps[:, :])
                lc0 = ep.tile((1, Bc), F32, name="lc0")
                cap0t = ep.tile((1, Bc), F32, name="cap0t")
                nc.sync.dma_start(cap0t[:, :], cap_stage[0:1, 7 * Bc:8 * Bc])
                nc.scalar.activation(lc0[:, :], cap0t[:, :], AF.Ln)
                nc.vector.tensor_tensor(lc0[:, :], lc0[:, :], ind_c0[:, :],
                                        OP.mult)
                nc.vector.tensor_tensor(fwd_sel[:, :], fwd_sel[:, :],
                                        lc0[:, :], OP.add)
                nc.vector.tensor_tensor(fwd_sel[:, :], fwd_sel[:, :],
                                        logsel[:, :], OP.add)
                shifts = ep.tile((1, Bc), F32, name="shifts")
                nc.vector.tensor_scalar(shifts[:, :], lenm1_row[:, :],
                                        A_SHIFT + MU, MU, OP.mult, OP.add)
                nc.vector.tensor_tensor(fwd_sel[:, :], fwd_sel[:, :],
                                        shifts[:, :], OP.add)
                fwd_tot = ep.tile((1, 1), F32, name="fwd_tot")
                nc.vector.tensor_reduce(fwd_tot[:, :], fwd_sel[:, :],
                                        mybir.AxisListType.X, OP.add)
                loss = ep.tile((1, 1), F32, name="loss")
                nc.vector.tensor_tensor(loss[:, :], fwd_tot[:, :],
                                        gold_ps[:, :], OP.subtract)
                nc.sync.dma_start(out_d.ap(), loss[:, :])
                if dbg:
                    gsb = ep.tile((1, 1), F32, name="gsb")
                    nc.scalar.copy(gsb[:, :], gold_ps[:, :])
                    fsel0 = ep.tile((1, Bc), F32, name="fsel0")
                    nc.scalar.copy(fsel0[:, :], fwd_ps[:, :])
                    nc.sync.dma_start(dbg_d.ap()[0:1, :], logsel[:, :])
                    nc.sync.dma_start(dbg_d.ap()[1:2, :], fwd_sel[:, :])
                    nc.sync.dma_start(dbg_d.ap()[2:3, :], lenm1_row[:, :])
                    nc.sync.dma_start(dbg_d.ap()[3:4, :], lc0[:, :])
                    nc.sync.dma_start(dbg_d.ap()[4:5, :], fsel0[:, :])
                    nc.sync.dma_start(dbg_d.ap()[5:6, 0:1], gsb[:, :])

    nc.compile()
    return nc


def shard_inputs(feats, transitions, start_transitions, end_transitions,
                 tags, mask, n_cores=N_CORES):
    feats = np.ascontiguousarray(np.asarray(feats, dtype=np.float32))
    transitions = np.ascontiguousarray(
        np.asarray(transitions, dtype=np.float32))
    start_transitions = np.ascontiguousarray(
        np.asarray(start_transitions, dtype=np.float32))
    end_transitions = np.ascontiguousarray(
        np.asarray(end_transitions, dtype=np.float32))
    tags = np.ascontiguousarray(np.asarray(tags).astype(np.int32))
    mask = np.ascontiguousarray(np.asarray(mask).astype(np.int32))
    Bc = feats.shape[0] // n_cores
    in_maps = []
    for c in range(n_cores):
        s = slice(c * Bc, (c + 1) * Bc)
        in_maps.append({
            "feats": feats[s],
            "trans": transitions,
            "start": start_transitions,
            "end": end_transitions,
            "tags": tags[s],
            "mask": mask[s],
        })
    return in_maps, feats.shape


def kernel(feats, transitions, start_transitions, end_transitions, tags,
           mask, **_ignored):
    in_maps, (Bf, L, _) = shard_inputs(
        feats, transitions, start_transitions, end_transitions, tags, mask)
    nc = build_program(L=L, Bc=Bf // N_CORES)
    res = run_bass_kernel_spmd(nc, in_maps, core_ids=list(range(N_CORES)))
    total = sum(float(r["out"][0, 0]) for r in res.results)
    return np.float32(total)

